# revision 42
# baseline (speedup 1.0000x reference)
"""nn_AttnBlock Trainium2 kernel (Bass/Tile), data-parallel over batch on 8 cores.

Contract: kernel(**inputs) takes the FULL unsharded inputs (as produced by
setup_inputs(): x [16,512,32,32] f32, gn_gamma/gn_beta [512], Wq/bq/Wk/bk/
Wv/bv/Wp/bp) and returns the FULL output [16,512,32,32] f32.

Strategy (per core = 2 samples; no cross-core communication):
  h   = GroupNorm(x)*gamma+beta              [c=512, t=1024] bf16, c on 4x128 partitions
  g   = (Wq^T Wk / sqrt(c))^T-applied: g = wm^T h        [c, i]
  vpT = (Wp Wv h)^T  (fused v+proj)                      [j, o']
  S^T = h^T g  -> E = exp(S^T)  (no max-subtraction; scores are O(1))
  Z   = ones^T E   (PE DoubleRow column sums, single PSUM bank, sequenced)
  p   = vpT^T E
  y   = p * (1/Z broadcast) + x   (+ Wp bv + bp via stt when nonzero)
All matmuls fp8 DoubleRow. The vpT matmuls are fused into the S loop sharing
each jo's lhsT (h slice) so their LDWEIGHTS dedup away, and they fill the
exp-evacuation-gated PE idle (keeps HAM warm). Evacuations are spread over
ACT (g, zrb, exp), DVE (vpT, tmp, h) and GPSIMD (y add; SBUF-only ops — Pool
cannot touch PSUM and rejects TensorScalarPtr on this HW).
The q/k fusion is exact when bq == bk == 0 (the spec fill); otherwise an
unfused variant with explicit q/k biases is built instead.
"""
import math
import sys
from contextlib import ExitStack

if "/opt/trn_rl_repo" not in sys.path:
    sys.path.insert(0, "/opt/trn_rl_repo")

import numpy as np
import ml_dtypes

import concourse.bass as bass
import concourse.tile as tile
from concourse import mybir
from concourse.bass_utils import run_bass_kernel_spmd

F32 = mybir.dt.float32
BF16 = mybir.dt.bfloat16

B = 16
C = 512
H = 32
W = 32
HW = H * W
NCHUNK = 4          # C / 128 partition chunks
NJT = 8             # HW / 128 key tiles
NNI = 2             # HW / 512 free-dim chunks
G = 16              # groups
EPS = 1e-6
N_CORES = 8
SPC = B // N_CORES  # samples per core


def _split_multiwait_drains(nc, max_waits=1):
    """walrus in this container rejects instructions carrying >1 sem waits
    ('Too many sync wait commands'); split extras into preceding single-wait
    Drain nops on the same engine."""
    f = nc.m.functions[0]
    ctr = 0
    for blk in f.blocks:
        insts = blk.instructions
        i = 0
        while i < len(insts):
            inst = insts[i]
            si = inst.sync_info
            waits = list(si.on_wait) if si and si.on_wait else []
            if len(waits) > max_waits:
                si.on_wait = waits[:max_waits]
                inst.sync_info = si
                for j, w in enumerate(waits[max_waits:]):
                    d = mybir.InstDrain(name=f"waitsplit_{ctr}", engine=inst.engine)
                    ctr += 1
                    d.sync_info = mybir.SyncInfo(on_wait=[w], on_update=[])
                    insts.insert(i + j, d)
                i += len(waits) - max_waits
            i += 1


def build_bf16(reps=1, fused=True):
    nc = bass.Bass("TRN2", target_bir_lowering=False, debug=False, num_devices=N_CORES)

    x_ext = nc.dram_tensor("x", [SPC, C, HW], F32, kind="ExternalInput").ap()
    wm_ext = nc.dram_tensor("wm", [C, C], BF16, kind="ExternalInput").ap()
    wvp_ext = nc.dram_tensor("wvp", [C, C], BF16, kind="ExternalInput").ap()
    if not fused:
        wk_ext = nc.dram_tensor("wk", [C, C], BF16, kind="ExternalInput").ap()
    # rows: gamma, beta, bpp, bq(scaled), bk
    vecs_ext = nc.dram_tensor("vecs", [5, C], F32, kind="ExternalInput").ap()
    mfw_ext = nc.dram_tensor("mask_fwd", [128, NCHUNK, G], F32, kind="ExternalInput").ap()
    mbw_ext = nc.dram_tensor("mask_bwd", [G, NCHUNK, 128], F32, kind="ExternalInput").ap()
    y_ext = nc.dram_tensor("y", [SPC, C, HW], F32, kind="ExternalOutput").ap()

    with tile.TileContext(nc) as tc, ExitStack() as ctx:
        pw = ctx.enter_context(tc.tile_pool(name="pw", bufs=1))
        px = ctx.enter_context(tc.tile_pool(name="px", bufs=2))
        ph = ctx.enter_context(tc.tile_pool(name="ph", bufs=2))
        pq = ctx.enter_context(tc.tile_pool(name="pq", bufs=2))
        pk = ctx.enter_context(tc.tile_pool(name="pk", bufs=2))
        pvt = ctx.enter_context(tc.tile_pool(name="pvt", bufs=2))
        pe = ctx.enter_context(tc.tile_pool(name="pe", bufs=2 if fused else 1))
        py = ctx.enter_context(tc.tile_pool(name="py", bufs=2))
        ptail = ctx.enter_context(tc.tile_pool(name="ptail", bufs=2))
        pzrb = ctx.enter_context(tc.tile_pool(name="pzrb", bufs=2))
        pzr = ctx.enter_context(tc.tile_pool(name="pzr", bufs=2))
        pg = ctx.enter_context(tc.tile_pool(name="pg", bufs=4))
        pp_big = ctx.enter_context(tc.tile_pool(name="pp_big", bufs=5, space="PSUM"))
        pp_z = ctx.enter_context(tc.tile_pool(name="pp_z", bufs=1, space="PSUM"))
        pp_small = ctx.enter_context(tc.tile_pool(name="pp_small", bufs=1, space="PSUM"))

        wm_sb = pw.tile([128, NCHUNK, C], BF16, tag="wm")
        nc.gpsimd.dma_start(out=wm_sb, in_=wm_ext.rearrange("(ci p) o -> p ci o", p=128))
        wvp_sb = pw.tile([128, NCHUNK, C], BF16, tag="wvp")
        nc.gpsimd.dma_start(out=wvp_sb, in_=wvp_ext.rearrange("(ci p) o -> p ci o", p=128))
        if not fused:
            wk_sb = pw.tile([128, NCHUNK, C], BF16, tag="wk")
            nc.gpsimd.dma_start(out=wk_sb, in_=wk_ext.rearrange("(ci p) o -> p ci o", p=128))
        vecs_sb = pw.tile([128, 5, NCHUNK], F32, tag="vecs")
        nc.gpsimd.dma_start(out=vecs_sb, in_=vecs_ext.rearrange("v (ci p) -> p v ci", p=128))
        gamma_sb = vecs_sb[:, 0, :]
        beta_sb = vecs_sb[:, 1, :]
        bpp_sb = vecs_sb[:, 2, :]
        bq_sb = vecs_sb[:, 3, :]
        bk_sb = vecs_sb[:, 4, :]
        mfw_sb = pw.tile([128, NCHUNK, G], F32, tag="mfw")
        nc.gpsimd.dma_start(out=mfw_sb, in_=mfw_ext)
        mbw_sb = pw.tile([G, NCHUNK, 128], F32, tag="mbw")
        nc.gpsimd.dma_start(out=mbw_sb, in_=mbw_ext)
        ones_sb = pw.tile([128, 1], BF16, tag="ones")
        nc.vector.memset(ones_sb, 1.0)
        onesf_sb = pw.tile([1, 128], F32, tag="onesf")
        nc.vector.memset(onesf_sb, 1.0)
        eps_sb = pw.tile([128, 1], F32, tag="eps")
        nc.vector.memset(eps_sb, EPS)

        for rep in range(reps):
            for s in range(SPC):
                x_t = px.tile([128, NCHUNK, HW], F32, tag="x")
                nc.sync.dma_start(
                    out=x_t, in_=x_ext[s].rearrange("(ci p) t -> p ci t", p=128)
                )

                # ---- GroupNorm stats: per-partition bn_stats, then group
                # aggregation via a tiny mask matmul; the group->channel
                # broadcast below uses a DMA gather (stride-0 inner dim).
                stat3 = pg.tile([128, NCHUNK, 3], F32, tag="stat3")
                for ci in range(NCHUNK):
                    st6 = pg.tile([128, 2, 6], F32, tag="st6")
                    for half in range(2):
                        nc.vector.bn_stats(
                            out=st6[:, half, :],
                            in_=x_t[:, ci, half * 512:(half + 1) * 512],
                        )
                    nc.vector.bn_aggr(out=stat3[:, ci, 0:2], in_=st6)
                    nc.vector.tensor_mul(
                        stat3[:, ci, 2:3], stat3[:, ci, 0:1], stat3[:, ci, 0:1]
                    )
                psum_g = pp_small.tile([G, 3], F32, tag="pssm")
                for ci in range(NCHUNK):
                    nc.tensor.matmul(
                        psum_g, mfw_sb[:, ci, :], stat3[:, ci, :],
                        start=(ci == 0), stop=(ci == NCHUNK - 1),
                    )
                gsb = pg.tile([G, 3], F32, tag="gsb")
                nc.scalar.activation(
                    out=gsb, in_=psum_g, func=mybir.ActivationFunctionType.Copy,
                )
                gs = pg.tile([G, 2], F32, tag="gs")
                t0 = pg.tile([G, 1], F32, tag="t0")
                nc.vector.tensor_mul(t0, gsb[:, 0:1], gsb[:, 0:1])
                nc.vector.tensor_add(gs[:, 1:2], gsb[:, 1:2], gsb[:, 2:3])
                nc.vector.tensor_sub(gs[:, 1:2], gs[:, 1:2], t0)
                # rsqrt(v+eps) = exp(-0.5*ln(v+eps)); Ln/Exp share an ACT table set
                nc.scalar.activation(
                    out=gs[:, 1:2], in_=gs[:, 1:2],
                    func=mybir.ActivationFunctionType.Ln,
                    bias=eps_sb[0:G], scale=1.0,
                )
                nc.scalar.activation(
                    out=gs[:, 1:2], in_=gs[:, 1:2],
                    func=mybir.ActivationFunctionType.Exp,
                    bias=0.0, scale=-0.5,
                )
                nc.vector.tensor_copy(out=gs[:, 0:1], in_=gsb[:, 0:1])

                # ---- normalize + affine -> h (bf16) ----
                h_t = ph.tile([128, NCHUNK, HW], BF16, tag="h")
                # group->channel broadcast via DMA gather (stride-0 inner dim)
                # instead of mask matmuls: saves PE work + PSUM contention
                bc_all = pg.tile([128, NCHUNK, 2], F32, tag="bc_all")
                for ci in range(NCHUNK):
                    sl = gs[4 * ci:4 * ci + 4, :]
                    src = bass.AP(tensor=sl.tensor, offset=sl.offset,
                                  ap=[list(sl.ap[0][:2]), [0, 32], [1, 2]])
                    nc.scalar.dma_start(out=bc_all[:, ci, :], in_=src)
                for ci in range(NCHUNK):
                    psum_bc = bc_all[:, ci, :]
                    A = pg.tile([128, 1], F32, tag="A")
                    Bt = pg.tile([128, 1], F32, tag="B")
                    nc.vector.tensor_mul(A, psum_bc[:, 1:2], gamma_sb[:, ci:ci + 1])
                    nc.vector.tensor_mul(Bt, psum_bc[:, 0:1], A)
                    nc.vector.tensor_sub(Bt, beta_sb[:, ci:ci + 1], Bt)
                    nc.vector.tensor_scalar(
                        out=h_t[:, ci, :], in0=x_t[:, ci, :],
                        scalar1=A, scalar2=Bt,
                        op0=mybir.AluOpType.mult, op1=mybir.AluOpType.add,
                    )

                # ---- g (fused: wm^T h == q with k-side folded) ----
                g_t = pq.tile([128, NCHUNK, HW], BF16, tag="g")
                for mo in range(NCHUNK):
                    for ni in range(NNI):
                        ps = pp_big.tile([128, 512], F32, tag="ps")
                        for ci in range(NCHUNK):
                            nc.tensor.matmul(
                                ps,
                                wm_sb[:, ci, mo * 128:(mo + 1) * 128],
                                h_t[:, ci, ni * 512:(ni + 1) * 512],
                                start=(ci == 0), stop=(ci == NCHUNK - 1),
                            )
                        if fused:
                            nc.scalar.activation(
                                out=g_t[:, mo, ni * 512:(ni + 1) * 512], in_=ps,
                                func=mybir.ActivationFunctionType.Copy,
                            )
                        else:
                            nc.scalar.activation(
                                out=g_t[:, mo, ni * 512:(ni + 1) * 512], in_=ps,
                                func=mybir.ActivationFunctionType.Identity,
                                bias=bq_sb[:, mo:mo + 1], scale=1.0,
                            )

                if not fused:
                    k_t = pk.tile([128, NCHUNK, HW], BF16, tag="k")
                    for mo in range(NCHUNK):
                        for ni in range(NNI):
                            ps = pp_big.tile([128, 512], F32, tag="ps")
                            for ci in range(NCHUNK):
                                nc.tensor.matmul(
                                    ps,
                                    wk_sb[:, ci, mo * 128:(mo + 1) * 128],
                                    h_t[:, ci, ni * 512:(ni + 1) * 512],
                                    start=(ci == 0), stop=(ci == NCHUNK - 1),
                                )
                            nc.scalar.activation(
                                out=k_t[:, mo, ni * 512:(ni + 1) * 512], in_=ps,
                                func=mybir.ActivationFunctionType.Identity,
                                bias=bk_sb[:, mo:mo + 1], scale=1.0,
                            )
                    s_lhs = k_t
                else:
                    s_lhs = h_t

                # ---- vpT = (Wp Wv h)^T : [j, o'] ----
                vpT_t = pvt.tile([128, NJT, C], BF16, tag="vpT")
                for jo in range(NJT):
                    ps = pp_big.tile([128, 512], F32, tag="ps")
                    for ci in range(NCHUNK):
                        nc.tensor.matmul(
                            ps,
                            h_t[:, ci, jo * 128:(jo + 1) * 128],
                            wvp_sb[:, ci, :],
                            start=(ci == 0), stop=(ci == NCHUNK - 1),
                        )
                    nc.scalar.activation(
                        out=vpT_t[:, jo, :], in_=ps,
                        func=mybir.ActivationFunctionType.Copy,
                    )

                # ---- S^T = s_lhs^T g, E = exp, Z column sums (PE) ----
                e_t = pe.tile([128, NJT, HW], BF16, tag="e")
                psz = pp_z.tile([64, 512], F32, tag="psz")
                for jo in range(NJT):
                    for ni in range(NNI):
                        ps = pp_big.tile([128, 512], F32, tag="ps")
                        for ci in range(NCHUNK):
                            nc.tensor.matmul(
                                ps,
                                s_lhs[:, ci, jo * 128:(jo + 1) * 128],
                                g_t[:, ci, ni * 512:(ni + 1) * 512],
                                start=(ci == 0), stop=(ci == NCHUNK - 1),
                            )
                        nc.scalar.activation(
                            out=e_t[:, jo, ni * 512:(ni + 1) * 512], in_=ps,
                            func=mybir.ActivationFunctionType.Exp,
                        )
                        nc.tensor.matmul(
                            psz[ni * 32:ni * 32 + 1, :], ones_sb,
                            e_t[:, jo, ni * 512:(ni + 1) * 512],
                            start=(jo == 0), stop=(jo == NJT - 1),
                        )

                # ---- 1/Z, broadcast across partitions via rank-1 matmul ----
                zr = pzr.tile([1, HW], F32, tag="zr", name=f"zr_{rep}_{s}")
                for ni in range(NNI):
                    nc.vector.reciprocal(
                        out=zr[:, ni * 512:(ni + 1) * 512],
                        in_=psz[ni * 32:ni * 32 + 1, :]
                    )
                zrb = pzrb.tile([128, HW], F32, tag="zrb", name=f"zrb_{rep}_{s}")
                for ni in range(NNI):
                    ps = pp_big.tile([128, 512], F32, tag="ps", name=f"zb_{rep}_{s}_{ni}")
                    nc.tensor.matmul(
                        ps, onesf_sb, zr[:, ni * 512:(ni + 1) * 512],
                        start=True, stop=True,
                    )
                    nc.scalar.activation(
                        out=zrb[:, ni * 512:(ni + 1) * 512], in_=ps,
                        func=mybir.ActivationFunctionType.Copy,
                    )

                # ---- p = vpT^T E; tail y = p*zr + (x + bpp) ----
                for mo in range(NCHUNK):
                    y_t = py.tile([128, HW], F32, tag="y")
                    for ni in range(NNI):
                        ps = pp_big.tile([128, 512], F32, tag="ps")
                        for jo in range(NJT):
                            nc.tensor.matmul(
                                ps,
                                vpT_t[:, jo, mo * 128:(mo + 1) * 128],
                                e_t[:, jo, ni * 512:(ni + 1) * 512],
                                start=(jo == 0), stop=(jo == NJT - 1),
                            )
                        tmp = ptail.tile([128, 512], F32, tag="tmp")
                        nc.vector.tensor_mul(tmp, ps, zrb[:, ni * 512:(ni + 1) * 512])
                        nc.vector.scalar_tensor_tensor(
                            out=y_t[:, ni * 512:(ni + 1) * 512],
                            in0=x_t[:, mo, ni * 512:(ni + 1) * 512],
                            scalar=bpp_sb[:, mo:mo + 1],
                            in1=tmp,
                            op0=mybir.AluOpType.add, op1=mybir.AluOpType.add,
                        )
                    nc.gpsimd.dma_start(
                        out=y_ext[s, mo * 128:(mo + 1) * 128, :], in_=y_t
                    )

    _split_multiwait_drains(nc)
    return nc


def make_host_inputs_bf16(x, gn_gamma, gn_beta, Wq, bq, Wk, bk, Wv, bv, Wp, bp, fused):
    scale = 1.0 / math.sqrt(C)
    wvp_l = ((Wp.astype(np.float64) @ Wv.astype(np.float64)).T).astype(ml_dtypes.bfloat16)
    bpp = (Wp.astype(np.float64) @ bv.astype(np.float64) + bp).astype(np.float32)
    if fused:
        # wm = Wq^T Wk / sqrt(c): S^T = (h^T wm^T) ... exact when bq=bk=0
        wm_l = (Wq.T.astype(np.float64) @ Wk.astype(np.float64) * scale).astype(ml_dtypes.bfloat16)
    else:
        wm_l = np.ascontiguousarray(Wq.T * scale).astype(ml_dtypes.bfloat16)
    wk_l = np.ascontiguousarray(Wk.T).astype(ml_dtypes.bfloat16)
    vecs = np.stack([
        gn_gamma, gn_beta, bpp, bq * scale, bk
    ]).astype(np.float32)

    grp = np.arange(C) // (C // G)
    mfw = np.zeros((128, NCHUNK, G), np.float32)
    mbw = np.zeros((G, NCHUNK, 128), np.float32)
    for ci in range(NCHUNK):
        for p in range(128):
            g = grp[ci * 128 + p]
            mfw[p, ci, g] = 1.0 / (C // G)
            mbw[g, ci, p] = 1.0

    xr = np.ascontiguousarray(x.reshape(B, C, HW)).astype(np.float32)
    in_maps = []
    for i in range(N_CORES):
        m = {
            "x": xr[i * SPC:(i + 1) * SPC],
            "wm": wm_l, "wvp": wvp_l,
            "vecs": vecs, "mask_fwd": mfw, "mask_bwd": mbw,
        }
        if not fused:
            m["wk"] = wk_l
        in_maps.append(m)
    return in_maps


_nc_cache = {}

FP8 = mybir.dt.float8e4

SW_M = 16.0          # host scale on wm
SW_VP = 16.0         # host scale on wvp
G_SCALE = 1.0 / 16.0  # device scale applied when evacuating g
S_E = 1.0 / (SW_M * G_SCALE * math.sqrt(C))
EB = -2.0            # exp bias (cancels in p/Z)

AF = mybir.ActivationFunctionType
OP = mybir.AluOpType
DR = mybir.MatmulPerfMode.DoubleRow

DEFAULT_CFG = dict(
    use_dr=True,       # fp8 DoubleRow matmuls (False: plain fp8, bf16 speed)
    psz_dr=False,      # DR for Z-sums (True costs an extra PSUM bank)
    psz_seq=True,      # DR Z-sums, single PSUM bank: ni=0 interleaved in the
                       # S loop, ni=1 sequenced between p-matmul chunks
    ablate="",         # comma-set of phases to skip (timing experiments only)
    h_eng="dve",       # h = x*A+B -> fp8 (emitted after attnB of prev iter)
    gevac_eng="act",   # g psum -> fp8 (scale 1/16)
    vpevac_eng="dve",  # vpT psum -> fp8 (NOTE: pool cannot read PSUM on HW;
                       # pool supports only tensor_tensor/copy, SBUF-only)
    tmp_eng="dve",     # tmp = p_psum * zrb
    stt_eng="dve",     # y = (x + bpp) + tmp
    tail_nobpp=True,   # bpp==0 fast tail: y = x + tmp via TensorTensor (DVE
                       # 2x mode); kernel() falls back to stt when bpp != 0
    yadd_eng="pool",   # engine for the nobpp y = x + tmp add (dve|pool)
    px_bufs=4,         # x_t double-buffer depth
    zrb_dma=False,     # broadcast 1/Z via gpsimd DMA instead of PE zps
                       # matmuls + ACT evac; saves 3 instructions but the DMA
                       # latency delays the tail ~1us/iter in sim -- off
    x_dtype="bf16",    # HBM transport dtype for x (residual precision ~0.2%)
    strip_ldw=False,   # drop separate InstLdweights -- BREAKS HW (NaN), keep off
    y_dtype="bf16",
)


def build_fp8(reps=1, cfg=None):
    cfg = dict(DEFAULT_CFG, **(cfg or {}))
    ab = set(x for x in cfg["ablate"].split(",") if x)
    ydt = F32 if cfg["y_dtype"] == "f32" else BF16
    xdt = F32 if cfg["x_dtype"] == "f32" else BF16

    nc = bass.Bass("TRN2", target_bir_lowering=False, debug=False,
                   num_devices=N_CORES)

    x_ext = nc.dram_tensor("x", [SPC, C, HW], xdt, kind="ExternalInput").ap()
    wm_ext = nc.dram_tensor("wm", [C, C], FP8, kind="ExternalInput").ap()
    wvp_ext = nc.dram_tensor("wvp", [C, C], FP8, kind="ExternalInput").ap()
    vecs_ext = nc.dram_tensor("vecs", [3, C], F32, kind="ExternalInput").ap()
    mfw_ext = nc.dram_tensor("mask_fwd", [128, NCHUNK, G], F32,
                             kind="ExternalInput").ap()
    y_ext = nc.dram_tensor("y", [SPC, C, HW], ydt, kind="ExternalOutput").ap()

    with tile.TileContext(nc) as tc, ExitStack() as ctx, \
            nc.allow_low_precision(reason="fp8 kernel; rel tolerance 2e-2"):
        pw = ctx.enter_context(tc.tile_pool(name="pw", bufs=1))
        px = ctx.enter_context(tc.tile_pool(name="px", bufs=int(cfg["px_bufs"])))
        ph = ctx.enter_context(tc.tile_pool(name="ph", bufs=3))
        pgt = ctx.enter_context(tc.tile_pool(name="pgt", bufs=2))
        pvt = ctx.enter_context(tc.tile_pool(name="pvt", bufs=2))
        pet = ctx.enter_context(tc.tile_pool(name="pet", bufs=2))
        pgn = ctx.enter_context(tc.tile_pool(name="pgn", bufs=3))
        pzr = ctx.enter_context(tc.tile_pool(name="pzr", bufs=3))
        pzrb = ctx.enter_context(tc.tile_pool(name="pzrb", bufs=3))
        ptmp = ctx.enter_context(tc.tile_pool(name="ptmp", bufs=5))
        py = ctx.enter_context(tc.tile_pool(name="py", bufs=2))
        psz_seq = cfg["use_dr"] and cfg["psz_seq"]
        psz_dr = cfg["use_dr"] and cfg["psz_dr"] and not psz_seq
        ppb = ctx.enter_context(tc.tile_pool(name="ppb", bufs=2 if psz_dr else 3,
                                             space="PSUM"))
        ppz = ctx.enter_context(tc.tile_pool(name="ppz", bufs=2 if psz_dr else 1,
                                             space="PSUM"))
        ppg = ctx.enter_context(tc.tile_pool(name="ppg", bufs=1, space="PSUM"))

        ENG = {"act": nc.scalar, "dve": nc.vector, "pool": nc.gpsimd}

        # ---- persistent weights/constants ----
        wm_sb = pw.tile([128, NCHUNK, C], FP8, tag="wm")
        nc.gpsimd.dma_start(out=wm_sb,
                            in_=wm_ext.rearrange("(p s) o -> p s o", p=128))
        wvp_sb = pw.tile([128, NCHUNK, C], FP8, tag="wvp")
        nc.gpsimd.dma_start(out=wvp_sb, in_=wvp_ext.rearrange("(p s) o -> p s o", p=128))
        vecs_sb = pw.tile([128, 3, NCHUNK], F32, tag="vecs")
        nc.gpsimd.dma_start(out=vecs_sb, in_=vecs_ext.rearrange("v (p s) -> p v s", p=128))
        gamma_sb = vecs_sb[:, 0, :]
        beta_sb = vecs_sb[:, 1, :]
        bpp_sb = vecs_sb[:, 2, :]
        mfw_sb = pw.tile([128, NCHUNK, G], F32, tag="mfw")
        nc.gpsimd.dma_start(out=mfw_sb, in_=mfw_ext)
        # DoubleRow lhsT needs k-pair stride %16==0 and >=32 dst partitions
        zrb_dma = cfg["zrb_dma"] and psz_seq  # DR Z path carries the scale
        ones8 = pw.tile([128, 2, 32], FP8, tag="ones8")
        # with zrb_dma the Z sums carry the SW_VP scale so 1/Z is 1/(16 Z)
        nc.vector.memset(ones8, SW_VP if zrb_dma else 1.0)
        ones32 = pw.tile([128, 32], FP8, tag="ones32")
        nc.vector.memset(ones32, 1.0)
        onesf = pw.tile([1, 128], BF16, tag="onesf")
        nc.vector.memset(onesf, 1.0 / SW_VP)
        eps_sb = pw.tile([128, 1], F32, tag="eps")
        nc.vector.memset(eps_sb, EPS)
        eb_sb = pw.tile([128, 1], F32, tag="eb")
        nc.vector.memset(eb_sb, EB)

        T = reps * SPC
        st = [dict() for _ in range(T)]

        def emit_xdma(it):
            x_t = px.tile([128, NCHUNK, HW], xdt, tag="x")
            nc.sync.dma_start(
                out=x_t, in_=x_ext[it % SPC].rearrange("(p s) t -> p s t", p=128))
            st[it]["x"] = x_t

        def emit_gn(it):
            x_t = st[it]["x"]
            if "gn" in ab:
                st[it]["bc"] = None
                return
            # per-(partition,chunk) stats
            stat3 = pgn.tile([128, NCHUNK, 3], F32, tag="stat3")
            for ci in range(NCHUNK):
                st6 = pgn.tile([128, 2, 6], F32, tag="st6")
                for half in range(2):
                    nc.vector.bn_stats(
                        out=st6[:, half, :],
                        in_=x_t[:, ci, half * 512:(half + 1) * 512])
                nc.vector.bn_aggr(out=stat3[:, ci, 0:2], in_=st6)
            nc.vector.tensor_mul(
                stat3[:, :, 2:3], stat3[:, :, 0:1], stat3[:, :, 0:1])
            # group aggregation (PE, tiny): [G,3] = mean, var, mean2 group-avgs
            psg = ppg.tile([G, 3], F32, tag="psg")
            for ci in range(NCHUNK):
                # mfw slices are identical per ci; one address dedups LDWs
                nc.tensor.matmul(psg, mfw_sb[:, 0, :], stat3[:, ci, :],
                                 start=(ci == 0), stop=(ci == NCHUNK - 1))
            gsb = pgn.tile([G, 3], F32, tag="gsb")
            nc.scalar.activation(out=gsb, in_=psg, func=AF.Copy)
            gs = pgn.tile([G, 2], F32, tag="gs")
            t0 = pgn.tile([G, 1], F32, tag="t0")
            nc.vector.tensor_mul(t0, gsb[:, 0:1], gsb[:, 0:1])
            nc.vector.tensor_add(gs[:, 1:2], gsb[:, 1:2], gsb[:, 2:3])
            nc.vector.tensor_sub(gs[:, 1:2], gs[:, 1:2], t0)
            # rsqrt(v+eps) = exp(-0.5*ln(v+eps))
            nc.scalar.activation(out=gs[:, 1:2], in_=gs[:, 1:2], func=AF.Ln,
                                 bias=eps_sb[0:G], scale=1.0)
            nc.scalar.activation(out=gs[:, 1:2], in_=gs[:, 1:2], func=AF.Exp,
                                 bias=0.0, scale=-0.5)
            nc.vector.tensor_copy(out=gs[:, 0:1], in_=gsb[:, 0:1])
            # group -> channel broadcast: channel 4p+s has group p//8, so one
            # gather replicates each group row to 8 partitions
            bc_all = pgn.tile([128, 2], F32, tag="bc_all")
            sl = gs[0:G, :]
            src = bass.AP(tensor=sl.tensor, offset=sl.offset,
                          ap=[list(sl.ap[0][:2]), [0, 8], [1, 2]])
            nc.gpsimd.dma_start(out=bc_all, in_=src)
            st[it]["bc"] = bc_all

        def emit_gn_post(it):
            # emitted after attnB(it-1): keeps DVE's recip/tail from queueing
            # behind the gather-DMA roundtrip
            bc_all = st[it]["bc"]
            if bc_all is None:
                A_all = pgn.tile([128, NCHUNK], F32, tag="A_all")
                B_all = pgn.tile([128, NCHUNK], F32, tag="B_all")
                nc.vector.memset(A_all, 1.0)
                nc.vector.memset(B_all, 0.0)
                st[it]["AB"] = (A_all, B_all)
                return
            A_all = pgn.tile([128, NCHUNK], F32, tag="A_all")
            B_all = pgn.tile([128, NCHUNK], F32, tag="B_all")
            nc.vector.tensor_scalar(out=A_all, in0=gamma_sb,
                                    scalar1=bc_all[:, 1:2], scalar2=None,
                                    op0=OP.mult)
            nc.vector.tensor_scalar(out=B_all, in0=A_all,
                                    scalar1=bc_all[:, 0:1], scalar2=None,
                                    op0=OP.mult)
            nc.vector.tensor_sub(B_all, beta_sb, B_all)
            st[it]["AB"] = (A_all, B_all)

        def emit_h(it):
            x_t = st[it]["x"]
            A_all, B_all = st[it]["AB"]
            # h = x*A + B -> fp8
            h_t = ph.tile([128, NCHUNK, HW], FP8, tag="h")
            he = ENG[cfg["h_eng"]]
            for ci in range(NCHUNK):
                if cfg["h_eng"] == "act":
                    he.activation(out=h_t[:, ci, :], in_=x_t[:, ci, :],
                                  func=AF.Identity,
                                  bias=B_all[:, ci:ci + 1], scale=A_all[:, ci:ci + 1])
                else:
                    he.tensor_scalar(out=h_t[:, ci, :], in0=x_t[:, ci, :],
                                     scalar1=A_all[:, ci:ci + 1],
                                     scalar2=B_all[:, ci:ci + 1],
                                     op0=OP.mult, op1=OP.add)
            st[it]["h"] = h_t

        def emit_vp_evac(it, jp):
            vpT_t = st[it]["vpT"]
            ve = ENG[cfg["vpevac_eng"]]
            ps = st[it]["vp_ps"]
            if cfg["vpevac_eng"] == "act":
                ve.activation(out=vpT_t[:, 2 * jp:2 * jp + 2, :],
                              in_=ps[:, 0:2, :], func=AF.Copy)
            else:
                ve.tensor_copy(out=vpT_t[:, 2 * jp:2 * jp + 2, :],
                               in_=ps[:, 0:2, :])

        def emit_attnA(it):
            h_t = st[it]["h"]
            st[it]["vpT"] = pvt.tile([128, NJT, C], FP8, tag="vpT",
                                     name=f"vpT_{it}")
            # ---- g = wm^T h (DoubleRow), evac *1/16 -> fp8 ----
            g_t = pgt.tile([128, NCHUNK, HW], FP8, tag="g")
            ge = ENG[cfg["gevac_eng"]]
            for mo in ([0] if "g" in ab else range(NCHUNK)):
                ps = ppb.tile([128, 2, 512], F32, tag="ps")
                if cfg["use_dr"]:
                    for cp in range(2):
                        for ni in range(NNI):
                            nc.tensor.matmul(
                                ps[:, ni, :],
                                wm_sb[:, 2 * cp:2 * cp + 2, mo * 128:(mo + 1) * 128],
                                h_t[:, 2 * cp:2 * cp + 2, ni * 512:(ni + 1) * 512],
                                start=(cp == 0), stop=(cp == 1), perf_mode=DR)
                else:
                    for ci in range(NCHUNK):
                        for ni in range(NNI):
                            nc.tensor.matmul(
                                ps[:, ni, :],
                                wm_sb[:, ci, mo * 128:(mo + 1) * 128],
                                h_t[:, ci, ni * 512:(ni + 1) * 512],
                                start=(ci == 0), stop=(ci == NCHUNK - 1))
                if cfg["gevac_eng"] == "act":
                    ge.activation(out=g_t[:, mo, :], in_=ps[:, 0:2, :],
                                  func=AF.Copy, scale=G_SCALE)
                else:
                    ge.tensor_scalar(out=g_t[:, mo, :], in0=ps[:, 0:2, :],
                                     scalar1=G_SCALE, scalar2=None, op0=OP.mult)
            st[it]["g"] = g_t

        def emit_attnB(it):
            x_t, h_t, g_t, vpT_t = (st[it][k] for k in ("x", "h", "g", "vpT"))
            # ---- S^T tiles + exp -> fp8 E; Z column sums on PE ----
            e_t = pet.tile([128, NJT, HW], FP8, tag="e")
            if psz_seq:
                # one [32,512] bank reused for both ni halves, sequenced
                psz1 = ppz.tile([32, 512], F32, tag="psz", name=f"psz_{it}")
                psz = [psz1, psz1]
            elif psz_dr:
                # DoubleRow dst must start at partition 0 -> one bank per ni
                psz = [ppz.tile([32, 512], F32, tag="psz", name=f"psz_{it}_{ni}")
                       for ni in range(NNI)]
            else:
                pszt = ppz.tile([64, 512], F32, tag="psz", name=f"psz_{it}")
                psz = [pszt[0:32, :], pszt[32:64, :]]
            jos = [0] if "s" in ab else list(range(NJT))
            for jo in jos:
                ps = ppb.tile([128, 2, 512], F32, tag="ps")
                do_vp = "vp" not in ab or jo < 2
                if do_vp and jo % 2 == 0:
                    st[it]["vp_ps"] = ppb.tile([128, 2, 512], F32, tag="ps",
                                               name=f"vpps_{it}_{jo // 2}")
                if cfg["use_dr"]:
                    for cp in range(2):
                        for ni in range(NNI):
                            nc.tensor.matmul(
                                ps[:, ni, :],
                                h_t[:, 2 * cp:2 * cp + 2, jo * 128:(jo + 1) * 128],
                                g_t[:, 2 * cp:2 * cp + 2, ni * 512:(ni + 1) * 512],
                                start=(cp == 0), stop=(cp == 1), perf_mode=DR)
                        if do_vp:
                            # same lhsT as the S matmuls above -> LDW dedups
                            nc.tensor.matmul(
                                st[it]["vp_ps"][:, jo % 2, :],
                                h_t[:, 2 * cp:2 * cp + 2, jo * 128:(jo + 1) * 128],
                                wvp_sb[:, 2 * cp:2 * cp + 2, :],
                                start=(cp == 0), stop=(cp == 1), perf_mode=DR)
                else:
                    for ci in range(NCHUNK):
                        for ni in range(NNI):
                            nc.tensor.matmul(
                                ps[:, ni, :],
                                h_t[:, ci, jo * 128:(jo + 1) * 128],
                                g_t[:, ci, ni * 512:(ni + 1) * 512],
                                start=(ci == 0), stop=(ci == NCHUNK - 1))
                        if do_vp:
                            nc.tensor.matmul(
                                st[it]["vp_ps"][:, jo % 2, :],
                                h_t[:, ci, jo * 128:(jo + 1) * 128],
                                wvp_sb[:, ci, :],
                                start=(ci == 0), stop=(ci == NCHUNK - 1))
                nc.scalar.activation(out=e_t[:, jo, :], in_=ps[:, 0:2, :],
                                     func=AF.Exp, bias=eb_sb, scale=S_E)
                if do_vp and jo % 2 == 1:
                    emit_vp_evac(it, jo // 2)
                if psz_seq and jo % 2 == 1:
                    a = jo // 2
                    nc.tensor.matmul(
                        psz[0], ones8, e_t[:, jo - 1:jo + 1, 0:512],
                        start=(a == 0), stop=(a == NJT // 2 - 1),
                        perf_mode=DR)
                elif psz_dr and jo % 2 == 1:
                    a = jo // 2
                    for ni in range(NNI):
                        nc.tensor.matmul(
                            psz[ni][:, :], ones8,
                            e_t[:, jo - 1:jo + 1, ni * 512:(ni + 1) * 512],
                            start=(a == 0), stop=(a == NJT // 2 - 1),
                            perf_mode=DR)
            if not psz_dr and not psz_seq and "psz" not in ab:
                # contiguous block: one ones32 load serves all 16 Z-sums
                for ni in range(NNI):
                    for jo in jos:
                        nc.tensor.matmul(
                            psz[ni][0:32, :], ones32,
                            e_t[:, jo, ni * 512:(ni + 1) * 512],
                            start=(jo == jos[0]), stop=(jo == jos[-1]))
            # ---- zr = 1/Z (bf16), zrb = broadcast(zr)/16 ----
            zr = pzr.tile([1, HW], BF16, tag="zr")
            if psz_seq:
                if "psz" not in ab:
                    nc.vector.reciprocal(out=zr[:, 0:512], in_=psz[0][0:1, :])
            else:
                for ni in ([] if "psz" in ab else range(NNI)):
                    nc.vector.reciprocal(out=zr[:, ni * 512:(ni + 1) * 512],
                                         in_=psz[ni][0:1, :])
            zps = None if zrb_dma else ppb.tile([128, 2, 512], F32, tag="ps")
            zrb = pzrb.tile([128, HW], BF16, tag="zrb")
            # ---- p = vpT^T E (DoubleRow); tail y = p*zrb + (x + bpp) ----
            s = it % SPC
            te = ENG[cfg["tmp_eng"]]
            se = ENG[cfg["stt_eng"]]
            y_t = py.tile([128, NCHUNK, HW], ydt, tag="y")
            for mo in range(NCHUNK):
                ps = ppb.tile([128, 2, 512], F32, tag="ps")
                if cfg["use_dr"]:
                    alist = [0] if "p" in ab else list(range(NJT // 2))
                    for a in alist:
                        for ni in range(NNI):
                            nc.tensor.matmul(
                                ps[:, ni, :],
                                vpT_t[:, 2 * a:2 * a + 2, mo * 128:(mo + 1) * 128],
                                e_t[:, 2 * a:2 * a + 2, ni * 512:(ni + 1) * 512],
                                start=(a == alist[0]), stop=(a == alist[-1]),
                                perf_mode=DR)
                else:
                    jlist = [0] if "p" in ab else list(range(NJT))
                    for jo in jlist:
                        for ni in range(NNI):
                            nc.tensor.matmul(
                                ps[:, ni, :],
                                vpT_t[:, jo, mo * 128:(mo + 1) * 128],
                                e_t[:, jo, ni * 512:(ni + 1) * 512],
                                start=(jo == jlist[0]), stop=(jo == jlist[-1]))
                if psz_seq and "psz" not in ab:
                    if mo == 0:
                        # Z sums for the second query half reuse the same
                        # bank; tile dep tracking orders them after recip0
                        for a in range(NJT // 2):
                            nc.tensor.matmul(
                                psz[1], ones8, e_t[:, 2 * a:2 * a + 2, 512:1024],
                                start=(a == 0), stop=(a == NJT // 2 - 1),
                                perf_mode=DR)
                        nc.vector.reciprocal(out=zr[:, 512:1024],
                                             in_=psz[1][0:1, :])
                    elif mo == 1:
                        if zrb_dma:
                            # replicate the single zr row across 128 dst
                            # partitions via a stride-0 free dim (same trick
                            # as the gn group broadcast gather)
                            src = bass.AP(tensor=zr.tensor, offset=zr.offset,
                                          ap=[list(zr.ap[0][:2]), [0, 128],
                                              [1, HW]])
                            nc.gpsimd.dma_start(out=zrb, in_=src)
                        else:
                            for ni in range(NNI):
                                nc.tensor.matmul(zps[:, ni, :], onesf,
                                                 zr[:, ni * 512:(ni + 1) * 512],
                                                 start=True, stop=True)
                            nc.scalar.activation(out=zrb, in_=zps[:, 0:2, :],
                                                 func=AF.Copy)
                elif psz_seq and mo == 0:
                    nc.vector.memset(zrb, 0.001)
                elif mo == 0 and "psz" not in ab:
                    # emit zrb matmuls after p(mo=0) is queued so PE doesn't
                    # stall on the DVE reciprocal
                    for ni in range(NNI):
                        nc.tensor.matmul(zps[:, ni, :], onesf,
                                         zr[:, ni * 512:(ni + 1) * 512],
                                         start=True, stop=True)
                    nc.scalar.activation(out=zrb, in_=zps[:, 0:2, :], func=AF.Copy)
                elif mo == 0:
                    nc.vector.memset(zrb, 0.001)

                def tail(mo, ps):
                    if "tail" in ab:
                        se.scalar_tensor_tensor(
                            out=y_t[:, mo, :], in0=x_t[:, mo, :],
                            scalar=bpp_sb[:, mo:mo + 1],
                            in1=zrb, op0=OP.add, op1=OP.add)
                        return
                    tmp = ptmp.tile([128, HW], BF16, tag="tmp")
                    te.tensor_mul(tmp, ps[:, 0:2, :], zrb)
                    if cfg["tail_nobpp"]:
                        ye = ENG[cfg["yadd_eng"]]
                        ye.tensor_add(y_t[:, mo, :], x_t[:, mo, :], tmp)
                    else:
                        se.scalar_tensor_tensor(
                            out=y_t[:, mo, :], in0=x_t[:, mo, :],
                            scalar=bpp_sb[:, mo:mo + 1],
                            in1=tmp, op0=OP.add, op1=OP.add)

                if psz_seq and mo == 0:
                    # zrb doesn't exist yet; run this tail after zrb (mo==1)
                    pend_tail = (mo, ps)
                else:
                    if psz_seq and mo == 1:
                        tail(*pend_tail)
                    tail(mo, ps)
            # Pool-issued DMA: keeps the y writeback off the SP queue so it
            # can't delay the x prefetches that feed the GN chain
            nc.gpsimd.dma_start(
                out=y_ext[it % SPC].rearrange("(p s) t -> p s t", p=128), in_=y_t)

        for w in range(min(3, T)):
            emit_xdma(w)
        emit_gn(0)
        emit_gn_post(0)
        emit_h(0)
        for it in range(T):
            if it + 3 < T:
                emit_xdma(it + 3)
            emit_attnA(it)
            if it + 1 < T:
                emit_gn(it + 1)
            emit_attnB(it)
            if it + 1 < T:
                emit_gn_post(it + 1)
                emit_h(it + 1)
            st[it] = None

    return nc


KERNEL_CFG = {}  # overrides for the fp8 build (tests only)


def _strip_all_ldweights(nc):
    """Remove every separate InstLdweights, folding its syncs into the next
    instruction — the InstMatmult still carries the weights AP (self-load)."""
    f = nc.m.functions[0]
    removed = 0
    for blk in f.blocks:
        out = []
        pend_w, pend_u = [], []
        for inst in blk.instructions:
            if inst.__class__.__name__ == "InstLdweights":
                si = inst.sync_info
                if si is not None:
                    pend_w += list(si.on_wait or [])
                    pend_u += list(si.on_update or [])
                removed += 1
                continue
            if pend_w or pend_u:
                si = inst.sync_info or mybir.SyncInfo(on_wait=[], on_update=[])
                si.on_wait = list(si.on_wait or []) + pend_w
                si.on_update = list(si.on_update or []) + pend_u
                inst.sync_info = si
                pend_w, pend_u = [], []
            out.append(inst)
        assert not (pend_w or pend_u)
        blk.instructions[:] = out
    return removed


def _dedup_ldweights(nc):
    """Remove InstLdweights whose weights AP matches the immediately
    preceding load on PE (only matmuls/drains in between): the PE array
    still holds those weights, so the reload is redundant. Waits/updates
    of removed loads migrate to the next instruction."""
    f = nc.m.functions[0]
    removed = 0
    for blk in f.blocks:
        insts = blk.instructions
        out = []
        last_sig = None
        pend_w, pend_u = [], []
        for inst in insts:
            kind = inst.__class__.__name__
            if str(inst.engine) != "EngineType.PE":
                out.append(inst)
                continue
            if kind == "InstLdweights":
                ap = inst.ins[0]
                sig = (ap.memref, ap.offset, str(ap.ap), str(ap.dtype),
                       str(inst.perf_mode), str(inst.is_transpose),
                       str(inst.tile_position), str(inst.tile_size))
                if sig == last_sig:
                    si = inst.sync_info
                    if si is not None:
                        pend_w += list(si.on_wait or [])
                        pend_u += list(si.on_update or [])
                    removed += 1
                    continue
                last_sig = sig
            elif kind not in ("InstMatmult", "InstDrain"):
                last_sig = None
            if pend_w or pend_u:
                si = inst.sync_info or mybir.SyncInfo(on_wait=[], on_update=[])
                si.on_wait = list(si.on_wait or []) + pend_w
                si.on_update = list(si.on_update or []) + pend_u
                inst.sync_info = si
                pend_w, pend_u = [], []
            out.append(inst)
        assert not (pend_w or pend_u)
        blk.instructions[:] = out
    return removed


def build(reps=1, fused=True, cfg=None):
    if fused:
        nc = build_fp8(reps=reps, cfg=cfg)
        if (cfg or {}).get("strip_ldw", DEFAULT_CFG.get("strip_ldw", True)):
            _strip_all_ldweights(nc)
        else:
            _dedup_ldweights(nc)
        _split_multiwait_drains(nc)
        return nc
    return build_bf16(reps=reps, fused=False)


def make_host_inputs(x, gn_gamma, gn_beta, Wq, bq, Wk, bk, Wv, bv, Wp, bp,
                     fused):
    if not fused:
        return make_host_inputs_bf16(
            x, gn_gamma, gn_beta, Wq, bq, Wk, bk, Wv, bv, Wp, bp, False)
    FP8NP = ml_dtypes.float8_e4m3

    def colperm(w):
        # output-channel k = s*128 + m holds original channel 4m+s, matching
        # the (p s) partition layout of x/h downstream
        return np.ascontiguousarray(
            w.reshape(-1, 128, 4).transpose(0, 2, 1).reshape(w.shape[0], -1))

    wm_l = colperm((Wq.T.astype(np.float64) @ Wk.astype(np.float64))
                   * SW_M).astype(FP8NP)
    wvp_l = colperm((Wp.astype(np.float64) @ Wv.astype(np.float64)).T
                    * SW_VP).astype(FP8NP)
    bpp = (Wp.astype(np.float64) @ bv.astype(np.float64) + bp).astype(np.float32)
    vecs = np.stack([gn_gamma, gn_beta, bpp]).astype(np.float32)

    # channel 4p+s -> group (4p+s)//32 == p//8 for every slot s
    mfw = np.zeros((128, NCHUNK, G), np.float32)
    for s in range(NCHUNK):
        for p in range(128):
            mfw[p, s, p // 8] = 1.0 / (C // G)

    xdt = (np.float32 if DEFAULT_CFG["x_dtype"] == "f32"
           else ml_dtypes.bfloat16)
    xr = np.ascontiguousarray(x.reshape(B, C, HW)).astype(xdt)
    return [{
        "x": xr[i * SPC:(i + 1) * SPC],
        "wm": wm_l, "wvp": wvp_l, "vecs": vecs,
        "mask_fwd": mfw,
    } for i in range(N_CORES)]


def kernel(x, gn_gamma, gn_beta, Wq, bq, Wk, bk, Wv, bv, Wp, bp):
    x = np.asarray(x, dtype=np.float32)
    args = {k: np.asarray(v, dtype=np.float32) for k, v in dict(
        gn_gamma=gn_gamma, gn_beta=gn_beta, Wq=Wq, bq=bq, Wk=Wk, bk=bk,
        Wv=Wv, bv=bv, Wp=Wp, bp=bp).items()}
    b, c, h, w = x.shape
    assert (b, c, h * w) == (B, C, HW), f"unexpected shape {x.shape}"

    # q/k fusion is exact only for zero q/k biases (the spec fill); fp8
    # scaling assumes O(1) data, so fall back to bf16 on extreme affine params
    fused = (np.abs(args["bq"]).max() == 0.0 and np.abs(args["bk"]).max() == 0.0
             and np.abs(args["gn_gamma"]).max() < 32.0
             and np.abs(args["gn_beta"]).max() < 32.0)

    cfg = dict(KERNEL_CFG)
    bpp_chk = args["Wp"].astype(np.float64) @ args["bv"].astype(np.float64) \
        + args["bp"]
    if np.abs(bpp_chk).max() != 0.0:
        cfg["tail_nobpp"] = False  # general path: y = (x + bpp) + tmp

    key = (fused, tuple(sorted(cfg.items())))
    if key not in _nc_cache:
        _nc_cache[key] = build(reps=1, fused=fused, cfg=cfg)
    nc = _nc_cache[key]

    in_maps = make_host_inputs(x, fused=fused, **args)
    res = run_bass_kernel_spmd(nc, in_maps, list(range(N_CORES)))
    y = np.concatenate([np.asarray(r["y"], np.float32) for r in res.results],
                       axis=0)
    return y.reshape(b, c, h, w).astype(np.float32)


if __name__ == "__main__":
    rng = np.random.default_rng(0)
    scale = 1.0 / math.sqrt(C)
    demo = dict(
        x=rng.standard_normal((B, C, H, W), dtype=np.float32),
        gn_gamma=np.ones(C, np.float32), gn_beta=np.zeros(C, np.float32),
        Wq=(rng.standard_normal((C, C)) * scale).astype(np.float32),
        bq=np.zeros(C, np.float32),
        Wk=(rng.standard_normal((C, C)) * scale).astype(np.float32),
        bk=np.zeros(C, np.float32),
        Wv=(rng.standard_normal((C, C)) * scale).astype(np.float32),
        bv=np.zeros(C, np.float32),
        Wp=(rng.standard_normal((C, C)) * scale).astype(np.float32),
        bp=np.zeros(C, np.float32),
    )
    out = kernel(**demo)
    print("kernel output:", out.shape, out.dtype, float(np.abs(out).max()))

